# revision 1
# baseline (speedup 1.0000x reference)
"""CARAFE content-aware upsampling kernel for Trainium2 (8 NeuronCores).

Problem: x(4,256,64,64) -> 1x1 down-conv(64ch) -> 3x3 enc-conv(100ch) ->
softmax over 25 reassembly taps -> content-aware reassembly + pixel shuffle
(x2) -> 1x1 out-conv(256ch).  Output (4,256,128,128).

Sharding: data-parallel over (batch n, H-half) = 8 shards; each core computes
32 output rows (64 upsampled rows) of one image.

Per-core algorithm (all layouts chosen so every contraction is a TensorE
matmul and the softmax runs along the free dimension):
  A) t = W_down@x + b_down          (64, 34, 68)   channels-on-partitions
  B) e = conv3x3(t) + b_enc         (100, 32*64)   via 9 shifted matmuls,
     PE-transpose 128-pixel slices -> softmax over 25 taps (pixels on
     partitions, taps strided in free dim) -> kern_T (pix, 100)
  C) y = W_out@x + b_out            computed at LOW resolution (the 1x1
     out-conv commutes with the reassembly because softmax weights sum to 1;
     zero-padded x makes the pad positions exactly b_out which keeps the
     identity exact at image borders), stored transposed (w', row, c) fp16
  D) reassembly per output row h: scatter the 100 softmax weights of the row
     into a banded matrix B[w', (dy,i,w,jj)] with gpsimd.local_scatter
     (per-partition indices), then out(c,(i,w,jj)) = sum_dy y_T(:,h+dy).T @ B_dy
     -- 5 PSUM-accumulated matmuls per c-half.  PSUM -> SBUF -> DMA with the
     free layout already in pixel-shuffled order.
"""
import sys

for _p in ("/opt/trn_rl_repo",):
    if _p not in sys.path:
        sys.path.insert(0, _p)

import numpy as np

N, C, H, W = 4, 256, 64, 64
D, KUP = 2, 5
CM, E, OC = 64, 100, 256
HH = 32          # output rows per core
RS = HH + 4      # x slab rows (2-halo each side)
TR = HH + 2      # t rows (1-halo each side)
WP = W + 4       # padded width

_CACHE = {}


def _scatter_index_table() -> np.ndarray:
    """Maps S positions (w', j*100+ch) -> column in the banded matrix B.

    S[w', j*100+ch] holds kern_T[pixel w'+j-4, ch] (5 partition-shifted
    copies of the row's softmax weights).  B[w', dy*256 + i*128 + w*2 + jj]
    must hold kern[(dy,dx), p=(i,jj), w] with w' = w + dx  (dx in 0..4).
    """
    sidx = np.full((80, 512), -1, np.int16)
    for wp in range(WP):
        for j in range(5):
            w = wp + j - 4
            if not (0 <= w < W):
                continue
            dxi = 4 - j
            for dy in range(5):
                for p in range(4):
                    i, jj = p // 2, p % 2
                    ch = (dy * 5 + dxi) * 4 + p
                    sidx[wp, j * 100 + ch] = dy * 256 + i * 128 + w * 2 + jj
    return sidx


def _build_program():
    if "nc" in _CACHE:
        return _CACHE["nc"]

    import concourse.bacc as bacc
    import concourse.mybir as mybir
    import concourse.tile as tile
    from concourse import bass

    F32, F16, I16 = mybir.dt.float32, mybir.dt.float16, mybir.dt.int16
    PSUM = bass.MemorySpace.PSUM

    nc = bacc.Bacc("TRN2", target_bir_lowering=False, debug=False, num_devices=8)

    xs_d = nc.dram_tensor("xs", [2, 128, RS, WP], F32, kind="ExternalInput")
    wd_d = nc.dram_tensor("wd", [2, 128, CM], F32, kind="ExternalInput")
    bd_d = nc.dram_tensor("bd", [1, CM], F32, kind="ExternalInput")
    we_d = nc.dram_tensor("we", [CM + 1, 9, E], F32, kind="ExternalInput")
    wo_d = nc.dram_tensor("wo", [2, 128, OC], F32, kind="ExternalInput")
    bo_d = nc.dram_tensor("bo", [1, OC], F32, kind="ExternalInput")
    vm_d = nc.dram_tensor("vm", [1, RS, WP], F32, kind="ExternalInput")
    id_d = nc.dram_tensor("idt", [128, 128], F32, kind="ExternalInput")
    si_d = nc.dram_tensor("six", [80, 512], I16, kind="ExternalInput")
    out_d = nc.dram_tensor("out", [2, 128, HH, 2, 128], F32, kind="ExternalOutput")

    with tile.TileContext(nc) as tc:
        with (
            tc.tile_pool(name="const", bufs=1) as cp,
            tc.tile_pool(name="esb", bufs=2) as ep_sb,
            tc.tile_pool(name="sm", bufs=2) as smp,
            tc.tile_pool(name="sS", bufs=3) as sp,
            tc.tile_pool(name="sB", bufs=3) as bp,
            tc.tile_pool(name="ro", bufs=3) as rop,
        ):
            xs0 = cp.tile([128, RS, WP], F32, tag="xs0")
            xs1 = cp.tile([128, RS, WP], F32, tag="xs1")
            wd0 = cp.tile([128, CM], F32, tag="wd0")
            wd1 = cp.tile([128, CM], F32, tag="wd1")
            bd_t = cp.tile([1, CM], F32, tag="bd")
            we_t = cp.tile([CM + 1, 9, E], F32, tag="we")
            wo0 = cp.tile([128, OC], F32, tag="wo0")
            wo1 = cp.tile([128, OC], F32, tag="wo1")
            bo_t = cp.tile([1, OC], F32, tag="bo")
            vm_t = cp.tile([1, RS, WP], F32, tag="vm")
            id_t = cp.tile([128, 128], F32, tag="idt")
            si_t = cp.tile([80, 512], I16, tag="six")
            one_t = cp.tile([1, WP], F32, tag="one")
            t_t = cp.tile([CM + 1, TR, WP], F32, tag="t")
            kern = cp.tile([128, 16, E], F16, tag="kern")
            yT = cp.tile([WP, RS, OC], F16, tag="yT")

            nc.sync.dma_start(xs0[:], xs_d[0])
            nc.sync.dma_start(xs1[:], xs_d[1])
            nc.sync.dma_start(wd0[:], wd_d[0])
            nc.sync.dma_start(wd1[:], wd_d[1])
            nc.sync.dma_start(bd_t[:], bd_d[:])
            nc.sync.dma_start(we_t[:], we_d[:])
            nc.sync.dma_start(wo0[:], wo_d[0])
            nc.sync.dma_start(wo1[:], wo_d[1])
            nc.sync.dma_start(bo_t[:], bo_d[:])
            nc.sync.dma_start(vm_t[:], vm_d[:])
            nc.sync.dma_start(id_t[:], id_d[:])
            nc.sync.dma_start(si_t[:], si_d[:])
            nc.vector.memset(one_t[:], 1.0)
            nc.vector.memset(t_t[CM : CM + 1, :, :], 1.0)

            # ---- phase A: t = W_down @ x + b_down (masked) ----
            with tc.tile_pool(name="tp", bufs=2, space=PSUM) as tpp:
                r0 = 0
                while r0 < TR:
                    nr = min(7, TR - r0)
                    tp = tpp.tile([CM, nr, WP], F32, tag="tp")
                    nc.tensor.matmul(tp[:], wd0[:], xs0[:, 1 + r0 : 1 + r0 + nr, :],
                                     start=True, stop=False)
                    nc.tensor.matmul(tp[:], wd1[:], xs1[:, 1 + r0 : 1 + r0 + nr, :],
                                     start=False, stop=False)
                    nc.tensor.matmul(tp[:], bd_t[:], vm_t[:, 1 + r0 : 1 + r0 + nr, :],
                                     start=False, stop=True)
                    nc.vector.tensor_copy(t_t[0:CM, r0 : r0 + nr, :], tp[:])
                    r0 += nr

            # ---- phase B: e = conv3x3(t) + b_enc, transpose, softmax ----
            with (
                tc.tile_pool(name="ep", bufs=2, space=PSUM) as epp,
                tc.tile_pool(name="etp", bufs=2, space=PSUM) as etpp,
            ):
                for chunk in range(4):
                    ep = epp.tile([E, 8, W], F32, tag="ep")
                    for tap in range(9):
                        dy, dx = tap // 3, tap % 3
                        nc.tensor.matmul(
                            ep[:],
                            we_t[:, tap, :],
                            t_t[:, 8 * chunk + dy : 8 * chunk + dy + 8, 1 + dx : 1 + dx + W],
                            start=(tap == 0), stop=(tap == 8),
                        )
                    es = ep_sb.tile([E, 8, W], F32, tag="es")
                    nc.vector.tensor_copy(es[:], ep[:])
                    for s in range(4):
                        etp = etpp.tile([128, E], F32, tag="etp")
                        nc.tensor.transpose(etp[:], es[:, 2 * s : 2 * s + 2, :],
                                            id_t[0:E, 0:E])
                        slot = kern[:, 4 * chunk + s, :]
                        nc.scalar.activation(slot, etp[:],
                                             mybir.ActivationFunctionType.Exp)
                        kv = slot.rearrange("p (k q) -> p q k", q=4)
                        ssum = smp.tile([128, 4, 1], F32, tag="ssum")
                        nc.vector.tensor_reduce(ssum[:], kv, mybir.AxisListType.X,
                                                mybir.AluOpType.add)
                        rinv = smp.tile([128, 4, 1], F32, tag="rinv")
                        nc.vector.reciprocal(rinv[:], ssum[:])
                        nc.vector.tensor_tensor(kv, kv, rinv[:].to_broadcast([128, 4, 25]),
                                                mybir.AluOpType.mult)

            # ---- phase C: y = W_out @ x + b_out, transposed fp16 ----
            with tc.tile_pool(name="yp", bufs=2, space=PSUM) as ypp:
                for r in range(RS):
                    yp = ypp.tile([WP, OC], F32, tag="yp")
                    nc.tensor.matmul(yp[:], xs0[:, r, :], wo0[:], start=True, stop=False)
                    nc.tensor.matmul(yp[:], xs1[:, r, :], wo1[:], start=False, stop=False)
                    nc.tensor.matmul(yp[:], one_t[:], bo_t[:], start=False, stop=True)
                    nc.vector.tensor_copy(yT[:, r, :], yp[:])

            # ---- phase D: banded reassembly, 5 matmuls per (h, c-half) ----
            with tc.tile_pool(name="rp", bufs=2, space=PSUM) as rpp:
                for h in range(HH):
                    S = sp.tile([80, 512], F16, tag="S")
                    src = kern[(h % 2) * 64 : (h % 2) * 64 + 64, h // 2, :]
                    for j in range(5):
                        # partition-shifted copies: DMA (engines can't start
                        # an access at an unaligned partition)
                        nc.sync.dma_start(S[4 - j : 68 - j, j * E : (j + 1) * E], src)
                    B = bp.tile([80, 1280], F16, tag="B")
                    nc.gpsimd.local_scatter(B[:], S[:], si_t[:],
                                            channels=80, num_elems=1280, num_idxs=512)
                    for cf in range(2):
                        rp = rpp.tile([128, 256], F32, tag="rp")
                        for dy in range(5):
                            nc.tensor.matmul(
                                rp[:],
                                yT[0:WP, h + dy, 128 * cf : 128 * (cf + 1)],
                                B[0:WP, 256 * dy : 256 * (dy + 1)],
                                start=(dy == 0), stop=(dy == 4),
                            )
                        ro = rop.tile([128, 2, 128], F32, tag="ro")
                        nc.vector.tensor_copy(ro[:], rp[:])
                        nc.sync.dma_start(out_d[cf, :, h, :, :], ro[:])

    nc.compile()
    _CACHE["nc"] = nc
    return nc


def _host_inputs(x, W_down, b_down, W_enc, b_enc, W_out, b_out):
    """Per-core input maps (core = 2*n + h_half)."""
    wd = np.ascontiguousarray(W_down.T.reshape(2, 128, CM), np.float32)
    bd = np.ascontiguousarray(b_down[None, :], np.float32)
    we = np.zeros((CM + 1, 9, E), np.float32)
    for tap in range(9):
        dy, dx = tap // 3, tap % 3
        we[:CM, tap, :] = W_enc[:, :, dy, dx].T
    we[CM, 4, :] = b_enc
    wo = np.ascontiguousarray(W_out.T.reshape(2, 128, OC), np.float32)
    bo = np.ascontiguousarray(b_out[None, :], np.float32)
    idt = np.eye(128, dtype=np.float32)
    six = _scatter_index_table()

    in_maps = []
    for core in range(8):
        n, h0 = core // 2, (core % 2) * HH
        xs = np.zeros((C, RS, WP), np.float32)
        vm = np.zeros((1, RS, WP), np.float32)
        lo, hi = max(0, h0 - 2), min(H, h0 + HH + 2)
        xs[:, lo - (h0 - 2) : hi - (h0 - 2), 2 : 2 + W] = x[n, :, lo:hi, :]
        vm[0, lo - (h0 - 2) : hi - (h0 - 2), 2 : 2 + W] = 1.0
        in_maps.append({
            "xs": xs.reshape(2, 128, RS, WP),
            "wd": wd, "bd": bd, "we": we, "wo": wo, "bo": bo,
            "vm": vm, "idt": idt, "six": six,
        })
    return in_maps


def kernel(x, W_down, b_down, W_enc, b_enc, W_out, b_out):
    from concourse.bass_utils import run_bass_kernel_spmd

    nc = _build_program()
    in_maps = _host_inputs(np.asarray(x, np.float32), np.asarray(W_down, np.float32),
                           np.asarray(b_down, np.float32), np.asarray(W_enc, np.float32),
                           np.asarray(b_enc, np.float32), np.asarray(W_out, np.float32),
                           np.asarray(b_out, np.float32))
    res = run_bass_kernel_spmd(nc, in_maps, list(range(8)))
    full = np.empty((N, C, 2 * H, 2 * W), np.float32)
    for core in range(8):
        n, half = core // 2, core % 2
        arr = res.results[core]["out"].reshape(C, HH * 2, 2 * W)
        full[n, :, half * 64 : (half + 1) * 64, :] = arr
    return full



# revision 3
# speedup vs baseline: 2.5939x; 2.5939x over previous
"""CARAFE content-aware upsampling kernel for Trainium2 (8 NeuronCores).

Problem: x(4,256,64,64) -> 1x1 down-conv(64ch) -> 3x3 enc-conv(100ch) ->
softmax over 25 reassembly taps -> content-aware reassembly + pixel shuffle
(x2) -> 1x1 out-conv(256ch).  Output (4,256,128,128).

Sharding: data-parallel over (batch n, H-half) = 8 shards; each core computes
32 output rows (64 upsampled rows) of one image.

Per-core algorithm (all matmul operands 16-bit so the PE runs 1 cycle/row;
fp32 runs 4 cycles/row):
  A) t = W_down@x + b_down          (64, 34, 68)   channels-on-partitions, bf16
  B) e = conv3x3(t) + b_enc         (100, 32*64)   via 9 shifted matmuls,
     PE-transpose 128-pixel slices -> softmax over 25 taps -> kern fp16
  C) y = W_out@x + b_out            computed at LOW resolution (the 1x1
     out-conv commutes with the reassembly because softmax weights sum to 1;
     zero-padded x makes the pad positions exactly b_out which keeps the
     identity exact at image borders), stored transposed (w', row, c) fp16
  D) reassembly per output row h: scatter the 100 softmax weights of the row
     into a banded matrix B[w', (dy,i,w,jj)] with gpsimd.local_scatter
     (per-partition indices), then out(c,(i,w,jj)) = sum_dy y_T(:,h+dy).T @ B_dy
     -- 5 PSUM-accumulated matmuls per c-half.  The 5 partition-shifted
     copies of kern feeding the scatter are built with 10 batched DMAs per
     8-row chunk (S_all) instead of 5 tiny DMAs per row.
"""
import sys

for _p in ("/opt/trn_rl_repo",):
    if _p not in sys.path:
        sys.path.insert(0, _p)

import numpy as np
import ml_dtypes

BF16 = ml_dtypes.bfloat16

N, C, H, W = 4, 256, 64, 64
D, KUP = 2, 5
CM, E, OC = 64, 100, 256
HH = 32          # output rows per core
RS = HH + 4      # x slab rows (2-halo each side)
TR = HH + 2      # t rows (1-halo each side)
WP = W + 4       # padded width

_CACHE = {}


def _scatter_index_table() -> np.ndarray:
    """Maps S positions (w', j*100+ch) -> column in the banded matrix B.

    S[w', j*100+ch] holds kern_T[pixel w'+j-4, ch] (5 partition-shifted
    copies of the row's softmax weights).  B[w', dy*256 + i*128 + w*2 + jj]
    must hold kern[(dy,dx), p=(i,jj), w] with w' = w + dx  (dx in 0..4).
    """
    sidx = np.full((80, 500), -1, np.int16)
    for wp in range(WP):
        for j in range(5):
            w = wp + j - 4
            if not (0 <= w < W):
                continue
            dxi = 4 - j
            for dy in range(5):
                for p in range(4):
                    i, jj = p // 2, p % 2
                    ch = (dy * 5 + dxi) * 4 + p
                    sidx[wp, j * 100 + ch] = dy * 256 + i * 128 + w * 2 + jj
    return sidx


def _build_program():
    if "nc" in _CACHE:
        return _CACHE["nc"]

    import concourse.bacc as bacc
    import concourse.mybir as mybir
    import concourse.tile as tile
    from concourse import bass

    F32, F16, B16, I16 = (mybir.dt.float32, mybir.dt.float16,
                          mybir.dt.bfloat16, mybir.dt.int16)
    PSUM = bass.MemorySpace.PSUM

    nc = bacc.Bacc("TRN2", target_bir_lowering=False, debug=False, num_devices=8)

    xs_d = nc.dram_tensor("xs", [2, 128, RS, WP], B16, kind="ExternalInput")
    wd_d = nc.dram_tensor("wd", [2, 128, CM], B16, kind="ExternalInput")
    bd_d = nc.dram_tensor("bd", [1, CM], B16, kind="ExternalInput")
    we_d = nc.dram_tensor("we", [CM + 1, 9, E], B16, kind="ExternalInput")
    wo_d = nc.dram_tensor("wo", [2, 128, OC], B16, kind="ExternalInput")
    bo_d = nc.dram_tensor("bo", [1, OC], B16, kind="ExternalInput")
    vm_d = nc.dram_tensor("vm", [1, RS, WP], B16, kind="ExternalInput")
    id_d = nc.dram_tensor("idt", [128, 128], B16, kind="ExternalInput")
    si_d = nc.dram_tensor("six", [80, 500], I16, kind="ExternalInput")
    out_d = nc.dram_tensor("out", [2, 128, HH, 2, 128], F32, kind="ExternalOutput")

    with tile.TileContext(nc) as tc:
        with (
            tc.tile_pool(name="const", bufs=1) as cp,
            tc.tile_pool(name="esb", bufs=2) as ep_sb,
            tc.tile_pool(name="sm", bufs=2) as smp,
            tc.tile_pool(name="sB", bufs=3) as bp,
            tc.tile_pool(name="ro", bufs=3) as rop,
        ):
            xs0 = cp.tile([128, RS, WP], B16, tag="xs0")
            xs1 = cp.tile([128, RS, WP], B16, tag="xs1")
            wd0 = cp.tile([128, CM], B16, tag="wd0")
            wd1 = cp.tile([128, CM], B16, tag="wd1")
            bd_t = cp.tile([1, CM], B16, tag="bd")
            we_t = cp.tile([CM + 1, 9, E], B16, tag="we")
            wo0 = cp.tile([128, OC], B16, tag="wo0")
            wo1 = cp.tile([128, OC], B16, tag="wo1")
            bo_t = cp.tile([1, OC], B16, tag="bo")
            vm_t = cp.tile([1, RS, WP], B16, tag="vm")
            id_t = cp.tile([128, 128], B16, tag="idt")
            si_t = cp.tile([80, 500], I16, tag="six")
            one_t = cp.tile([1, WP], B16, tag="one")
            t_t = cp.tile([CM + 1, TR, WP], B16, tag="t")
            kern = cp.tile([128, 16, E], F16, tag="kern")
            # 5 partition-shifted copies of kern rows: [w', hpair, row, j, ch]
            s_all = cp.tile([80, 16, 2, 5, E], F16, tag="sall")
            yT = cp.tile([WP, RS, OC], F16, tag="yT")

            nc.sync.dma_start(xs0[:], xs_d[0])
            nc.sync.dma_start(xs1[:], xs_d[1])
            nc.sync.dma_start(wd0[:], wd_d[0])
            nc.sync.dma_start(wd1[:], wd_d[1])
            nc.sync.dma_start(bd_t[:], bd_d[:])
            nc.sync.dma_start(we_t[:], we_d[:])
            nc.sync.dma_start(wo0[:], wo_d[0])
            nc.sync.dma_start(wo1[:], wo_d[1])
            nc.sync.dma_start(bo_t[:], bo_d[:])
            nc.sync.dma_start(vm_t[:], vm_d[:])
            nc.sync.dma_start(id_t[:], id_d[:])
            nc.sync.dma_start(si_t[:], si_d[:])
            nc.vector.memset(one_t[:], 1.0)
            nc.vector.memset(t_t[CM : CM + 1, :, :], 1.0)

            # ---- phase A: t = W_down @ x + b_down (masked) ----
            with tc.tile_pool(name="tp", bufs=2, space=PSUM) as tpp:
                r0 = 0
                while r0 < TR:
                    nr = min(7, TR - r0)
                    tp = tpp.tile([CM, nr, WP], F32, tag="tp")
                    nc.tensor.matmul(tp[:], wd0[:], xs0[:, 1 + r0 : 1 + r0 + nr, :],
                                     start=True, stop=False)
                    nc.tensor.matmul(tp[:], wd1[:], xs1[:, 1 + r0 : 1 + r0 + nr, :],
                                     start=False, stop=False)
                    nc.tensor.matmul(tp[:], bd_t[:], vm_t[:, 1 + r0 : 1 + r0 + nr, :],
                                     start=False, stop=True)
                    nc.vector.tensor_copy(t_t[0:CM, r0 : r0 + nr, :], tp[:])
                    r0 += nr

            # ---- phase B: e = conv3x3(t) + b_enc, transpose, softmax ----
            with (
                tc.tile_pool(name="ep", bufs=2, space=PSUM) as epp,
                tc.tile_pool(name="etp", bufs=2, space=PSUM) as etpp,
            ):
                for chunk in range(4):
                    ep = epp.tile([E, 8, W], F32, tag="ep")
                    for tap in range(9):
                        dy, dx = tap // 3, tap % 3
                        nc.tensor.matmul(
                            ep[:],
                            we_t[:, tap, :],
                            t_t[:, 8 * chunk + dy : 8 * chunk + dy + 8, 1 + dx : 1 + dx + W],
                            start=(tap == 0), stop=(tap == 8),
                        )
                    es = ep_sb.tile([E, 8, W], B16, tag="es")
                    nc.vector.tensor_copy(es[:], ep[:])
                    for s in range(4):
                        etp = etpp.tile([128, E], B16, tag="etp")
                        nc.tensor.transpose(etp[:], es[:, 2 * s : 2 * s + 2, :],
                                            id_t[0:E, 0:E])
                        slot = kern[:, 4 * chunk + s, :]
                        nc.scalar.activation(slot, etp[:],
                                             mybir.ActivationFunctionType.Exp)
                        kv = slot.rearrange("p (k q) -> p q k", q=4)
                        ssum = smp.tile([128, 4, 1], F32, tag="ssum")
                        nc.vector.tensor_reduce(ssum[:], kv, mybir.AxisListType.X,
                                                mybir.AluOpType.add)
                        rinv = smp.tile([128, 4, 1], F32, tag="rinv")
                        nc.vector.reciprocal(rinv[:], ssum[:])
                        nc.vector.tensor_tensor(kv, kv, rinv[:].to_broadcast([128, 4, 25]),
                                                mybir.AluOpType.mult)
                    # batched partition-shifted copies for this chunk's 8 rows
                    hp0 = 4 * chunk
                    for r in range(2):
                        for j in range(5):
                            nc.sync.dma_start(
                                s_all[4 - j : 68 - j, hp0 : hp0 + 4, r, j, :],
                                kern[64 * r : 64 * (r + 1), hp0 : hp0 + 4, :],
                            )

            # ---- phase C: y = W_out @ x + b_out, transposed fp16 ----
            with tc.tile_pool(name="yp", bufs=2, space=PSUM) as ypp:
                for r in range(RS):
                    yp = ypp.tile([WP, OC], F32, tag="yp")
                    nc.tensor.matmul(yp[:], xs0[:, r, :], wo0[:], start=True, stop=False)
                    nc.tensor.matmul(yp[:], xs1[:, r, :], wo1[:], start=False, stop=False)
                    nc.tensor.matmul(yp[:], one_t[:], bo_t[:], start=False, stop=True)
                    nc.vector.tensor_copy(yT[:, r, :], yp[:])

            # ---- phase D: banded reassembly, 5 matmuls per (h, c-half) ----
            with tc.tile_pool(name="rp", bufs=2, space=PSUM) as rpp:
                for h in range(HH):
                    B = bp.tile([80, 1280], F16, tag="B")
                    nc.gpsimd.local_scatter(B[:], s_all[:, h // 2, h % 2, :, :],
                                            si_t[:],
                                            channels=80, num_elems=1280, num_idxs=500)
                    for cf in range(2):
                        rp = rpp.tile([128, 256], F32, tag="rp")
                        for dy in range(5):
                            nc.tensor.matmul(
                                rp[:],
                                yT[0:WP, h + dy, 128 * cf : 128 * (cf + 1)],
                                B[0:WP, 256 * dy : 256 * (dy + 1)],
                                start=(dy == 0), stop=(dy == 4),
                            )
                        ro = rop.tile([128, 2, 128], F32, tag="ro")
                        nc.vector.tensor_copy(ro[:], rp[:])
                        nc.sync.dma_start(out_d[cf, :, h, :, :], ro[:])

    nc.compile()
    _CACHE["nc"] = nc
    return nc


def _host_inputs(x, W_down, b_down, W_enc, b_enc, W_out, b_out):
    """Per-core input maps (core = 2*n + h_half)."""
    wd = np.ascontiguousarray(W_down.T.reshape(2, 128, CM)).astype(BF16)
    bd = np.ascontiguousarray(b_down[None, :]).astype(BF16)
    we = np.zeros((CM + 1, 9, E), np.float32)
    for tap in range(9):
        dy, dx = tap // 3, tap % 3
        we[:CM, tap, :] = W_enc[:, :, dy, dx].T
    we[CM, 4, :] = b_enc
    we = we.astype(BF16)
    wo = np.ascontiguousarray(W_out.T.reshape(2, 128, OC)).astype(BF16)
    bo = np.ascontiguousarray(b_out[None, :]).astype(BF16)
    idt = np.eye(128, dtype=np.float32).astype(BF16)
    six = _scatter_index_table()

    in_maps = []
    for core in range(8):
        n, h0 = core // 2, (core % 2) * HH
        xs = np.zeros((C, RS, WP), np.float32)
        vm = np.zeros((1, RS, WP), np.float32)
        lo, hi = max(0, h0 - 2), min(H, h0 + HH + 2)
        xs[:, lo - (h0 - 2) : hi - (h0 - 2), 2 : 2 + W] = x[n, :, lo:hi, :]
        vm[0, lo - (h0 - 2) : hi - (h0 - 2), 2 : 2 + W] = 1.0
        in_maps.append({
            "xs": xs.reshape(2, 128, RS, WP).astype(BF16),
            "wd": wd, "bd": bd, "we": we, "wo": wo, "bo": bo,
            "vm": vm.astype(BF16), "idt": idt, "six": six,
        })
    return in_maps


def kernel(x, W_down, b_down, W_enc, b_enc, W_out, b_out):
    from concourse.bass_utils import run_bass_kernel_spmd

    nc = _build_program()
    in_maps = _host_inputs(np.asarray(x, np.float32), np.asarray(W_down, np.float32),
                           np.asarray(b_down, np.float32), np.asarray(W_enc, np.float32),
                           np.asarray(b_enc, np.float32), np.asarray(W_out, np.float32),
                           np.asarray(b_out, np.float32))
    res = run_bass_kernel_spmd(nc, in_maps, list(range(8)))
    full = np.empty((N, C, 2 * H, 2 * W), np.float32)
    for core in range(8):
        n, half = core // 2, core % 2
        arr = res.results[core]["out"].reshape(C, HH * 2, 2 * W)
        full[n, :, half * 64 : (half + 1) * 64, :] = arr
    return full


# revision 7
# speedup vs baseline: 2.6940x; 1.0386x over previous
"""CARAFE content-aware upsampling kernel for Trainium2 (8 NeuronCores).

Problem: x(4,256,64,64) -> 1x1 down-conv(64ch) -> 3x3 enc-conv(100ch) ->
softmax over 25 reassembly taps -> content-aware reassembly + pixel shuffle
(x2) -> 1x1 out-conv(256ch).  Output (4,256,128,128).

Sharding: data-parallel over (batch n, H-half) = 8 shards; each core computes
32 output rows (64 upsampled rows) of one image.

All matmul operands are 16-bit (1 PE cycle/row; fp32 runs 4).  The PE row
count is the bottleneck, so the reassembly packs the dy-taps into the
contraction dimension:

  A) t = W_down@x + b_down         bf16 (64, 34, 68)
  B) e = conv3x3(t) + b_enc        9 taps as (dy01-pair, dy2+bias) matmuls
     against a row-shifted copy of t -> softmax over 25 taps -> kern fp16
  C) y = W_out@x + b_out           low-res (commutes with reassembly since
     softmax weights sum to 1; zero-padded x + bias row keeps borders exact),
     bias added on the PSUM->SBUF copy; yT[w', row, c] fp16
  D) out[pix,c](h,wh) = sum_{(dy,u)} B5[(dy,u),pix] * Y5[(dy,u),h,c]
     with w-halves wh (u in [0,36) covers a 32+4 window) and dy packed in
     partitions as two groups g0={0,1,2} (108 par), g1={3,4} (72 par):
     2 PSUM-accumulated matmuls of 256 rows per (h,wh).
     Y5 = 20 shifted DMA copies of yT.  B5 via gpsimd.local_scatter from S5;
     S5 (5 j-shifted, dy-replicated copies of kern rows) is built with 40
     one-hot shift matmuls (PE moves data across partitions).
"""
import sys

for _p in ("/opt/trn_rl_repo",):
    if _p not in sys.path:
        sys.path.insert(0, _p)

import numpy as np
import ml_dtypes

BF16 = ml_dtypes.bfloat16
F16 = np.float16

N, C, H, W = 4, 256, 64, 64
D, KUP = 2, 5
CM, E, OC = 64, 100, 256
HH = 32          # output rows per core
RS = HH + 4      # x slab rows (2-halo each side)
TR = HH + 2      # t rows (1-halo each side)
WP = W + 4       # padded width
NH = 8           # rows per scatter call
GDY = (3, 2)     # dy group sizes
GCH = (112, 80)  # scatter channels (dy-group partitions padded to %16)

_CACHE = {}


def _scatter_tables():
    """idx[g][part, hi, j, dy', p] -> hi*128 + pix, or -1.

    Partition part = dyl*36 + u holds S5 values kern[w_row = 32wh+u+j-4, ch]
    for all (j, dy', p).  Element (j, dy', p) lands at pix = i*64 + w_loc*2
    + jj (w_loc = u-4+j) iff dy' == dyl + goff and w_loc in [0,32).
    """
    tabs = []
    goff = 0
    for g, gd in enumerate(GDY):
        t = np.full((GCH[g], NH, 5, 5, 4), -1, np.int16)
        for dyl in range(gd):
            for u in range(36):
                part = dyl * 36 + u
                for hi in range(NH):
                    for j in range(5):
                        w_loc = u - 4 + j
                        if not (0 <= w_loc < 32):
                            continue
                        for p in range(4):
                            i, jj = p // 2, p % 2
                            t[part, hi, j, dyl + goff, p] = (
                                hi * 128 + i * 64 + w_loc * 2 + jj)
        tabs.append(t.reshape(GCH[g], NH * 100))
        goff += gd
    return tabs


def _shift_mats():
    """sh[g][wh, j, r, 128, M] one-hot: col dyl*36+u hot at row r*64 + w_row,
    w_row = 32wh + u + j - 4 (when in [0,64))."""
    mats = []
    for g, gd in enumerate(GDY):
        m = np.zeros((2, 5, 2, 128, gd * 36), F16)
        for wh in range(2):
            for j in range(5):
                for r in range(2):
                    for dyl in range(gd):
                        for u in range(36):
                            w_row = 32 * wh + u + j - 4
                            if 0 <= w_row < 64:
                                m[wh, j, r, r * 64 + w_row, dyl * 36 + u] = 1.0
        mats.append(m)
    return mats


def _build_program():
    if "nc" in _CACHE:
        return _CACHE["nc"]

    import concourse.bacc as bacc
    import concourse.mybir as mybir
    import concourse.tile as tile
    from concourse import bass

    F32, FP16, B16, I16 = (mybir.dt.float32, mybir.dt.float16,
                           mybir.dt.bfloat16, mybir.dt.int16)
    PSUM = bass.MemorySpace.PSUM

    nc = bacc.Bacc("TRN2", target_bir_lowering=False, debug=False, num_devices=8)

    xs_d = nc.dram_tensor("xs", [2, 128, RS, WP], B16, kind="ExternalInput")
    wd_d = nc.dram_tensor("wd", [2, 128, CM], B16, kind="ExternalInput")
    bd_d = nc.dram_tensor("bd", [1, CM], B16, kind="ExternalInput")
    # we2: taps for (dy0,dy1) pairs: [3dx, 128, E]; we1: dy2 taps + bias [65, 3dx, E]
    we2_d = nc.dram_tensor("we2", [128, 3, E], B16, kind="ExternalInput")
    we1_d = nc.dram_tensor("we1", [CM + 1, 3, E], B16, kind="ExternalInput")
    wo_d = nc.dram_tensor("wo", [2, 128, OC], B16, kind="ExternalInput")
    vm_d = nc.dram_tensor("vm", [1, RS, WP], B16, kind="ExternalInput")
    id_d = nc.dram_tensor("idt", [128, 128], B16, kind="ExternalInput")
    si0_d = nc.dram_tensor("six0", [GCH[0], NH * 100], I16, kind="ExternalInput")
    si1_d = nc.dram_tensor("six1", [GCH[1], NH * 100], I16, kind="ExternalInput")
    sh0_d = nc.dram_tensor("sh0", [128, 2, 5, 2, 108], FP16, kind="ExternalInput")
    sh1_d = nc.dram_tensor("sh1", [128, 2, 5, 2, 72], FP16, kind="ExternalInput")
    out_d = nc.dram_tensor("out", [HH, 128, 2, OC], F32, kind="ExternalOutput")

    with tile.TileContext(nc) as tc:
        with (
            tc.tile_pool(name="const", bufs=1) as cp,
            tc.tile_pool(name="esb", bufs=2) as ep_sb,
            tc.tile_pool(name="sm", bufs=2) as smp,
            tc.tile_pool(name="ro", bufs=3) as rop,
        ):
            xs0 = cp.tile([128, RS, WP], B16, tag="xs0")
            xs1 = cp.tile([128, RS, WP], B16, tag="xs1")
            wd0 = cp.tile([128, CM], B16, tag="wd0")
            wd1 = cp.tile([128, CM], B16, tag="wd1")
            bd_t = cp.tile([1, CM], B16, tag="bd")
            we2_t = cp.tile([128, 3, E], B16, tag="we2")
            we1_t = cp.tile([CM + 1, 3, E], B16, tag="we1")
            wo0 = cp.tile([128, OC], B16, tag="wo0")
            wo1 = cp.tile([128, OC], B16, tag="wo1")
            vm_t = cp.tile([1, RS, WP], B16, tag="vm")
            id_t = cp.tile([128, 128], B16, tag="idt")
            si_t = [cp.tile([GCH[0], NH * 100], I16, tag="six0", name="six0"),
                    cp.tile([GCH[1], NH * 100], I16, tag="six1", name="six1")]
            sh_t = [cp.tile([128, 2, 5, 2, 108], FP16, tag="sh0", name="sh0"),
                    cp.tile([128, 2, 5, 2, 72], FP16, tag="sh1", name="sh1")]
            t_t = cp.tile([CM + 1, TR, WP], B16, tag="t")
            t2_t = cp.tile([128, TR - 1, WP], B16, tag="t2")
            kern = cp.tile([128, 16, E], FP16, tag="kern")
            yT = cp.tile([WP, RS, OC], FP16, tag="yT")
            # S5/B5/Y5 per (group, w-half)
            s5 = [[cp.tile([GCH[g], HH, 5, 5, 4], FP16, tag=f"s5_{g}{wh}", name=f"s5_{g}{wh}")
                   for wh in range(2)] for g in range(2)]
            b5 = [[cp.tile([GCH[g], HH, 128], FP16, tag=f"b5_{g}{wh}", name=f"b5_{g}{wh}")
                   for wh in range(2)] for g in range(2)]
            y5 = [[cp.tile([GCH[g], HH, OC], FP16, tag=f"y5_{g}{wh}", name=f"y5_{g}{wh}")
                   for wh in range(2)] for g in range(2)]

            nc.sync.dma_start(xs0[:], xs_d[0])
            nc.sync.dma_start(xs1[:], xs_d[1])
            nc.sync.dma_start(wd0[:], wd_d[0])
            nc.sync.dma_start(wd1[:], wd_d[1])
            nc.sync.dma_start(bd_t[:], bd_d[:])
            nc.sync.dma_start(we2_t[:], we2_d[:])
            nc.sync.dma_start(we1_t[:], we1_d[:])
            nc.sync.dma_start(wo0[:], wo_d[0])
            nc.sync.dma_start(wo1[:], wo_d[1])
            nc.sync.dma_start(vm_t[:], vm_d[:])
            nc.sync.dma_start(id_t[:], id_d[:])
            nc.scalar.dma_start(si_t[0][:], si0_d[:])
            nc.scalar.dma_start(si_t[1][:], si1_d[:])
            nc.scalar.dma_start(sh_t[0][:], sh0_d[:])
            nc.scalar.dma_start(sh_t[1][:], sh1_d[:])
            nc.vector.memset(t_t[CM : CM + 1, :, :], 1.0)

            # ---- phase A: t = W_down @ x + b_down (masked) ----
            with tc.tile_pool(name="tp", bufs=2, space=PSUM) as tpp:
                r0 = 0
                while r0 < TR:
                    nr = min(7, TR - r0)
                    tp = tpp.tile([CM, nr, WP], F32, tag="tp")
                    nc.tensor.matmul(tp[:], wd0[:], xs0[:, 1 + r0 : 1 + r0 + nr, :],
                                     start=True, stop=False)
                    nc.tensor.matmul(tp[:], wd1[:], xs1[:, 1 + r0 : 1 + r0 + nr, :],
                                     start=False, stop=False)
                    nc.tensor.matmul(tp[:], bd_t[:], vm_t[:, 1 + r0 : 1 + r0 + nr, :],
                                     start=False, stop=True)
                    nc.vector.tensor_copy(t_t[0:CM, r0 : r0 + nr, :], tp[:])
                    r0 += nr
            # row-shifted copy for the dy01 pair matmuls: t2[0:64,ri]=t[ri],
            # t2[64:128,ri]=t[ri+1]
            nc.scalar.dma_start(t2_t[0:CM, :, :], t_t[0:CM, 0 : TR - 1, :])
            nc.scalar.dma_start(t2_t[CM:128, :, :], t_t[0:CM, 1:TR, :])

            # ---- phase B: e = conv3x3(t) + b_enc, transpose, softmax ----
            with (
                tc.tile_pool(name="ep", bufs=2, space=PSUM) as epp,
                tc.tile_pool(name="etp", bufs=2, space=PSUM) as etpp,
            ):
                for chunk in range(4):
                    ep = epp.tile([E, 8, W], F32, tag="ep")
                    for dx in range(3):
                        nc.tensor.matmul(
                            ep[:],
                            we2_t[:, dx, :],
                            t2_t[:, 8 * chunk : 8 * chunk + 8, 1 + dx : 1 + dx + W],
                            start=(dx == 0), stop=False,
                        )
                    for dx in range(3):
                        nc.tensor.matmul(
                            ep[:],
                            we1_t[:, dx, :],
                            t_t[:, 8 * chunk + 2 : 8 * chunk + 10, 1 + dx : 1 + dx + W],
                            start=False, stop=(dx == 2),
                        )
                    es = ep_sb.tile([E, 8, W], B16, tag="es")
                    nc.vector.tensor_copy(es[:], ep[:])
                    for s in range(4):
                        etp = etpp.tile([128, E], B16, tag="etp")
                        nc.tensor.transpose(etp[:], es[:, 2 * s : 2 * s + 2, :],
                                            id_t[0:E, 0:E])
                        slot = kern[:, 4 * chunk + s, :]
                        nc.scalar.activation(slot, etp[:],
                                             mybir.ActivationFunctionType.Exp)
                        kv = slot.rearrange("p (k q) -> p q k", q=4)
                        ssum = smp.tile([128, 4, 1], F32, tag="ssum")
                        nc.vector.tensor_reduce(ssum[:], kv, mybir.AxisListType.X,
                                                mybir.AluOpType.add)
                        rinv = smp.tile([128, 4, 1], F32, tag="rinv")
                        nc.vector.reciprocal(rinv[:], ssum[:])
                        nc.vector.tensor_tensor(kv, kv, rinv[:].to_broadcast([128, 4, 25]),
                                                mybir.AluOpType.mult)

            # ---- S5 build: 40 one-hot shift matmuls over kern ----
            # rhs AP: kern channels (dy*5+dxi)*4+q viewed as [p, hp, dxi, dy, q]
            kern_v = kern[:].rearrange("p hp (dy dxi q) -> p hp dxi dy q", dy=5, dxi=5, q=4)
            with tc.tile_pool(name="s5p", bufs=3, space=PSUM) as s5pp:
                for g in range(2):
                    m = GDY[g] * 36
                    for wh in range(2):
                        s5v = s5[g][wh][:].rearrange("c (hp r) j d q -> c hp r j d q", r=2)
                        for j in range(5):
                            for r in range(2):
                                sp = s5pp.tile([m, 16, 5, 4], F32, tag="s5p")
                                nc.tensor.matmul(sp[:], sh_t[g][:, wh, j, r, :],
                                                 kern_v[:, :, 4 - j, :, :],
                                                 start=True, stop=True)
                                nc.vector.tensor_copy(s5v[0:m, :, r, j, :, :], sp[:])

            # ---- B5 scatter (gpsimd), NH rows per call ----
            for hb in range(HH // NH):
                for g in range(2):
                    for wh in range(2):
                        nc.gpsimd.local_scatter(
                            b5[g][wh][:, hb * NH : (hb + 1) * NH, :],
                            s5[g][wh][:, hb * NH : (hb + 1) * NH, :, :, :],
                            si_t[g][:],
                            channels=GCH[g], num_elems=NH * 128, num_idxs=NH * 100)

            # ---- phase C: y = W_out @ x, bias on copy-out; yT fp16 ----
            with tc.tile_pool(name="yp", bufs=2, space=PSUM) as ypp:
                for r in range(RS):
                    yp = ypp.tile([WP, OC], F32, tag="yp")
                    nc.tensor.matmul(yp[:], xs0[:, r, :], wo0[:], start=True, stop=False)
                    nc.tensor.matmul(yp[:], xs1[:, r, :], wo1[:], start=False, stop=True)
                    nc.vector.tensor_copy(yT[:, r, :], yp[:])

            # ---- Y5: shifted copies of yT rows, h-halves for pipelining ----
            for hh in range(2):
                for g in range(2):
                    goff = 0 if g == 0 else 3
                    for dyl in range(GDY[g]):
                        for wh in range(2):
                            nc.scalar.dma_start(
                                y5[g][wh][dyl * 36 : dyl * 36 + 36,
                                          hh * 16 : hh * 16 + 16, :],
                                yT[32 * wh : 32 * wh + 36,
                                   dyl + goff + hh * 16 : dyl + goff + hh * 16 + 16, :])

            # ---- phase D: 2 matmuls per (h, wh), out[pix, c] ----
            with tc.tile_pool(name="rp", bufs=4, space=PSUM) as rpp:
                for h in range(HH):
                    ro = rop.tile([128, 2, OC], F32, tag="ro")
                    for wh in range(2):
                        rp = rpp.tile([128, OC], F32, tag="rp")
                        nc.tensor.matmul(rp[:], b5[0][wh][0:108, h, :],
                                         y5[0][wh][0:108, h, :],
                                         start=True, stop=False)
                        nc.tensor.matmul(rp[:], b5[1][wh][0:72, h, :],
                                         y5[1][wh][0:72, h, :],
                                         start=False, stop=True)
                        if wh == 0:
                            nc.vector.tensor_copy(ro[:, wh, :], rp[:])
                        else:
                            nc.scalar.copy(ro[:, wh, :], rp[:])
                    nc.sync.dma_start(out_d[h], ro[:])

    nc.compile()
    _CACHE["nc"] = nc
    return nc


def _host_inputs(x, W_down, b_down, W_enc, b_enc, W_out, b_out):
    """Per-core input maps (core = 2*n + h_half)."""
    wd = np.ascontiguousarray(W_down.T.reshape(2, 128, CM)).astype(BF16)
    bd = np.ascontiguousarray(b_down[None, :]).astype(BF16)
    we2 = np.zeros((128, 3, E), np.float32)
    we1 = np.zeros((CM + 1, 3, E), np.float32)
    for dx in range(3):
        we2[0:CM, dx, :] = W_enc[:, :, 0, dx].T
        we2[CM:128, dx, :] = W_enc[:, :, 1, dx].T
        we1[0:CM, dx, :] = W_enc[:, :, 2, dx].T
    we1[CM, 1, :] = b_enc
    wo = np.ascontiguousarray(W_out.T.reshape(2, 128, OC)).astype(BF16)
    idt = np.eye(128, dtype=np.float32).astype(BF16)
    six = _scatter_tables()
    sh = _shift_mats()

    in_maps = []
    for core in range(8):
        n, h0 = core // 2, (core % 2) * HH
        xs = np.zeros((C, RS, WP), np.float32)
        vm = np.zeros((1, RS, WP), np.float32)
        lo, hi = max(0, h0 - 2), min(H, h0 + HH + 2)
        xs[:, lo - (h0 - 2) : hi - (h0 - 2), 2 : 2 + W] = x[n, :, lo:hi, :]
        vm[0, lo - (h0 - 2) : hi - (h0 - 2), 2 : 2 + W] = 1.0
        in_maps.append({
            "xs": xs.reshape(2, 128, RS, WP).astype(BF16),
            "wd": wd, "bd": bd,
            "we2": we2.astype(BF16), "we1": we1.astype(BF16),
            "wo": wo,
            "vm": vm.astype(BF16), "idt": idt,
            "six0": six[0], "six1": six[1],
            "sh0": np.ascontiguousarray(sh[0].transpose(3, 0, 1, 2, 4)),
            "sh1": np.ascontiguousarray(sh[1].transpose(3, 0, 1, 2, 4)),
        })
    return in_maps


def kernel(x, W_down, b_down, W_enc, b_enc, W_out, b_out):
    from concourse.bass_utils import run_bass_kernel_spmd

    nc = _build_program()
    in_maps = _host_inputs(np.asarray(x, np.float32), np.asarray(W_down, np.float32),
                           np.asarray(b_down, np.float32), np.asarray(W_enc, np.float32),
                           np.asarray(b_enc, np.float32), np.asarray(W_out, np.float32),
                           np.asarray(b_out, np.float32))
    res = run_bass_kernel_spmd(nc, in_maps, list(range(8)))
    full = np.empty((N, C, 2 * H, 2 * W), np.float32)
    for core in range(8):
        n, half = core // 2, core % 2
        # out[h, (i, wl, jj), wh, c] -> (c, 2h+i, 64wh + 2wl + jj)
        arr = res.results[core]["out"].reshape(HH, 2, 32, 2, 2, OC)
        arr = arr.transpose(5, 0, 1, 4, 2, 3).reshape(OC, 2 * HH, 2 * W)
        full[n, :, half * 64 : (half + 1) * 64, :] = arr
    full += np.asarray(b_out, np.float32)[None, :, None, None]
    return full
